# revision 16
# baseline (speedup 1.0000x reference)
"""Trainium2 Bass kernel for nn_AttentionBasisSynthesizer.

out[b] = softmax(Q[b] @ K[b].T + bias) @ V[b], bias[k] built from a tiny
sinusoidal atom bank (computed on host, replicated to every core).

Sharding: data-parallel over the batch dim — 8 batches onto 8 NeuronCores,
one batch per core. Each core computes its full [S, S] attention.

Device-side layout (per core): scores are computed TRANSPOSED, sT[k, q], so
- the key-dim bias is per-partition and folds into the ACT engine's free
  exp(scale*x + bias) affine,
- P @ V needs no transposes: out.T[d, q] = sum over k-tiles of
  matmul(lhsT=V_tile[k,d], rhs=exp_scores[k, q]).
Softmax uses a constant shift C instead of a per-row max (mathematically
exact; scores ~ N(0, sqrt(128)) so exp(s - C) can neither overflow nor
flush the row maximum for any plausible data).
The denominator Z[q] = sum_k p[k, q] is a partition-axis sum: p-tiles are
pairwise-folded in bf16 on the Vector engine (cheap 2x mode), and the last
128-partition sum is a ones-matmul on the Tensor engine into PSUM.
"""

import numpy as np

import concourse.bass as bass
import concourse.tile as tile
from concourse import mybir
from concourse.bass_utils import run_bass_kernel_spmd
from concourse.vector_clock import ScopedClock

B, S, D = 8, 2048, 128
KT = S // 128          # 16 key tiles of 128
NCH = S // 512         # 4 psum chunks of 512
C_SHIFT = 64.0         # constant softmax shift (exact: softmax(x-C)=softmax(x))

F32 = mybir.dt.float32
F32R = mybir.dt.float32r
BF16 = mybir.dt.bfloat16
EXP = mybir.ActivationFunctionType.Exp


def _install_tile_drain_patch():
    """This container's walrus accepts only one semaphore wait per sync-queue
    instruction, but TileContext's tail drain carries one wait per tracked
    proc. Split the waits across single-wait NOPs ahead of the drain (the
    sync queue is in-order, so the drain still begins only after every wait
    has been satisfied)."""

    def _drain_and_barrier(self, tick_clock, wait_clock):
        nc = self.nc
        probe = nc.sync.nop()
        wait_clock.add_sem_waits(
            probe.ins, ScopedClock({None: tick_clock.global_clock})
        )
        si = probe.ins.sync_info
        waits = list(si.on_wait or []) if si is not None else []
        if len(waits) > 1:
            si.on_wait = waits[:1]
            for w in waits[1:]:
                extra = nc.sync.nop()
                extra.ins.sync_info = mybir.SyncInfo(on_wait=[w], on_update=[])
        nc.sync.drain()
        nc.all_engine_barrier()
        assert self.sems is not None
        popped = nc._tile_sem_poison_stack.pop()
        assert popped is self._sem_poison
        nc.clear_and_free_semaphores(list(self.sems.allocated().values()))
        nc.all_engine_barrier()

    tile.TileContext._drain_and_barrier = _drain_and_barrier


def _split_multi_waits(nc: bass.Bass, limit: int = 1) -> int:
    """This container's walrus rejects instructions carrying more than one
    semaphore wait ("Too many sync wait commands"). Hoist excess waits onto
    same-engine NOPs inserted immediately before the instruction — engine
    queues dispatch in order, so the semantics are identical."""
    n_split = 0
    for fn in nc.m.functions:
        for blk in fn.blocks:
            insts = blk.instructions
            out = []
            for inst in insts:
                si = inst.sync_info
                waits = list(si.on_wait or []) if si is not None else []
                if len(waits) > limit:
                    keep = waits[:limit]
                    extra = waits[limit:]
                    for j in range(0, len(extra), limit):
                        nop = mybir.InstNoOp(
                            name=f"{inst.name}-waitsplit{j}",
                            ins=[],
                            outs=[],
                            engine=inst.engine,
                        )
                        nop.sync_info = mybir.SyncInfo(
                            on_wait=extra[j : j + limit], on_update=[]
                        )
                        nc.register_instruction(nop, overwrite=True)
                        out.append(nop)
                        n_split += 1
                    si.on_wait = keep
                out.append(inst)
            if n_split:
                blk.instructions = out
    return n_split


def build_nc() -> bass.Bass:
    _install_tile_drain_patch()
    nc = bass.Bass()

    qT = nc.declare_dram_parameter("qT", [D, S], F32R, isOutput=False)
    kT = nc.declare_dram_parameter("kT", [D, S], F32R, isOutput=False)
    v = nc.declare_dram_parameter("v", [S, D], F32, isOutput=False)
    biasb = nc.declare_dram_parameter("biasb", [128, KT], F32, isOutput=False)
    oT = nc.declare_dram_parameter("oT", [D, S], F32, isOutput=True)

    with tile.TileContext(nc) as tc:
        with (
            tc.tile_pool(name="const", bufs=1) as const,
            tc.tile_pool(name="pp", bufs=5) as pp,
            tc.tile_pool(name="l1p", bufs=3) as l1p,
            tc.tile_pool(name="l2p", bufs=3) as l2p,
            tc.tile_pool(name="l3p", bufs=3) as l3p,
            tc.tile_pool(name="l4p", bufs=2) as l4p,
            tc.tile_pool(name="outp", bufs=1) as outp,
            tc.tile_pool(name="sps", bufs=2, space="PSUM") as sps,
            tc.tile_pool(name="ops", bufs=1, space="PSUM") as ops,
        ):
            kTs = const.tile([D, S], F32R)
            qTs = const.tile([D, S], F32R)
            bias_s = const.tile([128, KT], F32)
            ones_s = const.tile([128, 128], BF16)
            vstage = const.tile([128, KT, D], F32)
            vb = const.tile([128, KT, D], BF16)

            # DMA order matters: queue issue costs ~0.6us per dma_start, so
            # few, large DMAs — split only where the pipeline needs early
            # availability (kT tile 0 + the first qT half feed QK(0)). The
            # v/bias loads ride the SWDGE queue in parallel.
            nc.gpsimd.dma_start(bias_s[:], biasb[:])
            nc.vector.memset(ones_s[:], 1.0)
            nc.sync.dma_start(kTs[:, 0:128], kT[:, 0:128])
            nc.sync.dma_start(qTs[:, 0:512], qT[:, 0:512])
            nc.sync.dma_start(qTs[:, 512:1024], qT[:, 512:1024])
            nc.sync.dma_start(kTs[:, 128:512], kT[:, 128:512])
            nc.sync.dma_start(qTs[:, 1024:2048], qT[:, 1024:2048])
            nc.sync.dma_start(kTs[:, 512:2048], kT[:, 512:2048])
            for g in range(NCH):
                ki0 = g * 4
                src = v[ki0 * 128 : (ki0 + 4) * 128, :].rearrange(
                    "(t p) d -> p t d", p=128
                )
                nc.gpsimd.dma_start(vstage[:, ki0 : ki0 + 4, :], src)
                nc.vector.tensor_copy(vb[:, ki0 : ki0 + 4, :],
                                      vstage[:, ki0 : ki0 + 4, :])

            o_ps = ops.tile([128, S], F32, tag="o")

            # Z fold tree operates on [128, 1024] halves so the tail levels
            # pipeline behind the exp halves instead of serializing after the
            # last full tile. The tree is imbalanced: p0..p7 and p8..p13 are
            # pre-folded during the loop, so only two add-levels remain after
            # the final exp ((p14+p15), then acc).
            HV = 2  # halves
            p_tiles = []
            pools = {1: (l1p, "l1"), 2: (l2p, "l2"), 3: (l3p, "l3"), 4: (l4p, "l4")}

            def fold(level, parent_pair, name):
                pool, tag = pools[level]
                t = pool.tile([128, S], BF16, tag=tag, name=name)
                a, b_ = parent_pair
                for h in range(HV):
                    sl = slice(h * (S // HV), (h + 1) * (S // HV))
                    nc.vector.tensor_add(t[:, sl], a[:, sl], b_[:, sl])
                return t

            folds = {}

            for ki in range(KT):
                p_t = pp.tile([128, S], BF16, tag="p")
                for h in range(2):
                    sp = sps.tile([128, 1024], F32, tag="sp")
                    for c in range(2):
                        q0 = c * 512
                        nc.tensor.matmul(
                            sp[:, q0 : q0 + 512],
                            lhsT=kTs[:, ki * 128 : (ki + 1) * 128],
                            rhs=qTs[:, h * 1024 + q0 : h * 1024 + q0 + 512],
                            start=True,
                            stop=True,
                        )
                    nc.scalar.activation(
                        p_t[:, h * 1024 : (h + 1) * 1024],
                        sp[:],
                        EXP,
                        bias=bias_s[:, ki : ki + 1],
                        scale=1.0,
                    )
                # PV accumulation into oT psum
                for c in range(NCH):
                    nc.tensor.matmul(
                        o_ps[:, c * 512 : (c + 1) * 512],
                        lhsT=vb[:, ki, :],
                        rhs=p_t[:, c * 512 : (c + 1) * 512],
                        start=(ki == 0),
                        stop=(ki == KT - 1),
                    )
                # Z folding tree (bf16, DVE 2x mode), imbalanced for a short
                # tail: each entry is (result_range, left_range, right_range);
                # a range of a single index means the raw p tile.
                p_tiles.append(p_t)
                plan = {
                    1: [((0, 1), (0, 0), (1, 1))],
                    3: [((2, 3), (2, 2), (3, 3)), ((0, 3), (0, 1), (2, 3))],
                    5: [((4, 5), (4, 4), (5, 5))],
                    7: [((6, 7), (6, 6), (7, 7)), ((4, 7), (4, 5), (6, 7)),
                        ((0, 7), (0, 3), (4, 7))],
                    9: [((8, 9), (8, 8), (9, 9))],
                    11: [((10, 11), (10, 10), (11, 11)),
                         ((8, 11), (8, 9), (10, 11))],
                    13: [((12, 13), (12, 12), (13, 13)),
                         ((8, 13), (8, 11), (12, 13)),
                         ((0, 13), (0, 7), (8, 13))],
                    15: [((14, 15), (14, 14), (15, 15)),
                         ((0, 15), (0, 13), (14, 15))],
                }

                def get(rng):
                    return p_tiles[rng[0]] if rng[0] == rng[1] else folds[rng]

                for rng, left, right in plan.get(ki, []):
                    size = rng[1] - rng[0] + 1
                    lvl = {2: 1, 4: 2, 6: 3, 8: 3, 14: 4, 16: 4}[size]
                    folds[rng] = fold(
                        lvl, (get(left), get(right)), f"f{rng[0]}_{rng[1]}"
                    )

            acc = folds[(0, 15)]

            # Tail: per-512-chunk Z ones-matmul -> reciprocal -> normalize ->
            # store, interleaved so DVE/PE/DMA overlap.
            rz = outp.tile([128, S], F32, tag="rz")
            oTs = outp.tile([128, S], F32, tag="oTs")
            zts = [
                sps.tile([128, 1024], F32, tag="sp", name=f"zt{i}")
                for i in range(2)
            ]
            for c in range(NCH):
                zt = zts[c // 2]
                z0 = (c % 2) * 512
                sl = slice(c * 512, (c + 1) * 512)
                nc.tensor.matmul(
                    zt[:, z0 : z0 + 512],
                    lhsT=ones_s[:],
                    rhs=acc[:, sl],
                    start=True,
                    stop=True,
                )
                nc.vector.reciprocal(rz[:, sl], zt[:, z0 : z0 + 512])
                nc.vector.tensor_mul(oTs[:, sl], o_ps[:, sl], rz[:, sl])
                nc.sync.dma_start(oT[:, sl], oTs[:, sl])

    _split_multi_waits(nc)
    return nc


def _bias_kernel(waveforms, gains, window, atom_indices, shifts) -> np.ndarray:
    waveforms = np.asarray(waveforms, dtype=np.float32)
    gains = np.asarray(gains, dtype=np.float32)
    window = np.asarray(window, dtype=np.float32)
    atom_indices = np.asarray(atom_indices).astype(np.int64)
    shifts = np.asarray(shifts).astype(np.int64)
    atoms = waveforms[atom_indices, :S]                  # [P, S]
    bases = atoms * gains[:, None]                       # [P, S]
    shifted = np.stack(
        [np.roll(bases[p], shifts[p]) for p in range(bases.shape[0])]
    )
    return (shifted * window[None, :S]).sum(0).astype(np.float32)  # [S]


def kernel(queries, keys, values, waveforms, gains, window, atom_indices, shifts):
    queries = np.asarray(queries, dtype=np.float32)
    keys = np.asarray(keys, dtype=np.float32)
    values = np.asarray(values, dtype=np.float32)

    bias = _bias_kernel(waveforms, gains, window, atom_indices, shifts)
    biasb = np.ascontiguousarray((bias - C_SHIFT).reshape(KT, 128).T)  # [128, KT]

    nc = build_nc()
    in_maps = [
        {
            "qT": np.ascontiguousarray(queries[b].T),
            "kT": np.ascontiguousarray(keys[b].T),
            "v": np.ascontiguousarray(values[b]),
            "biasb": biasb,
        }
        for b in range(B)
    ]
    res = run_bass_kernel_spmd(nc, in_maps, list(range(B)))
    out = np.stack([np.ascontiguousarray(res.results[b]["oT"].T) for b in range(B)])
    return out.astype(np.float32)
